# revision 21
# baseline (speedup 1.0000x reference)
"""Trainium2 Bass kernel for a dense transformer block (pre-LN, causal MHA + 4x MLP).

Sharding over 8 NeuronCores: attention is head-sharded 8 ways (each core does
H/8 heads for BOTH batches — identical causal structure on every core), then a
single 8-rank AllToAll re-shards activations to (batch, token-block) shards so
the out-projection and MLP run row-sharded with full weights (no all-reduce).

All on-chip activations are kept feature-major ("transposed": features on the
partition axis) so matmuls consume weights in their natural [in, out] layout
and no on-chip transposes are needed. The host supplies x pre-transposed and
re-transposes the output. LayerNorm statistics (partition-axis reductions) are
computed with ones-vector matmuls on the TensorEngine; softmax denominators
ride along as an appended ones-column in the attention-value matmul.

Matmuls run as float32r (fp32 storage, single-pass PE mode) for 4x throughput
over plain fp32.
"""
import numpy as np
from contextlib import ExitStack

import concourse.bass as bass
import concourse.mybir as mybir
import concourse.tile as tile
from concourse import bacc

F32 = mybir.dt.float32
F32R = mybir.dt.float32r
AF = mybir.ActivationFunctionType
ALU = mybir.AluOpType


class Cfg:
    def __init__(self, D=1024, DFF=4096, H=16, T=2048, B=2, TP=4, HD=64,
                 mm_fast=True, gelu_native=True):
        self.gelu_native = gelu_native
        self.D, self.DFF, self.H, self.T, self.B, self.TP, self.HD = D, DFF, H, T, B, TP, HD
        self.NC = B * TP              # cores
        self.mm_fast = mm_fast
        self.KD = D // 128            # feature chunks
        self.FD = DFF // 128          # hidden chunks
        self.TCH = min(512, T)        # attention q-block width
        self.LCH = min(512, T)        # LN/QKV streaming token chunk
        self.NTL = T // self.LCH      # streaming chunks per batch
        self.NT = T // self.TCH       # token chunks per batch
        self.QB = self.NT             # attention q blocks per batch
        self.NKC = T // 128           # key chunks per batch
        self.NDIAG = self.TCH // 128  # diagonal masks
        self.NHC = H // self.NC       # heads per core (8-way head shard)
        self.HC = self.NHC * HD       # head feature columns per core
        self.HPT = min(2, self.NHC)   # heads per SBUF tile
        self.PT = self.HPT * HD       # partitions per head tile (64 or 128)
        self.HT = self.NHC // self.HPT  # head tiles per batch
        self.TQ = T // TP             # owned tokens per core
        self.OCB = min(512, D)        # output-feature column block
        self.HCB = min(512, DFF)      # hidden column block
        assert H % self.NC == 0 and T % TP == 0 and D % 128 == 0
        assert HD == 64 and self.TCH % 128 == 0 and DFF % 128 == 0


CFG = Cfg()


def emit(ctx: ExitStack, tc: tile.TileContext, io: dict, cfg: Cfg):
    nc = tc.nc
    c = cfg
    rearr = lambda ap: ap.rearrange("(o p) t -> p o t", p=128)

    def mm(ps, lhsT, rhs, start, stop):
        if c.mm_fast:
            lhsT, rhs = lhsT.bitcast(F32R), rhs.bitcast(F32R)
        nc.tensor.matmul(ps, lhsT, rhs, start=start, stop=stop)

    # writes into matmul-feeding tiles must be rounded to f32r
    rnd = (lambda ap: ap.bitcast(F32R)) if c.mm_fast else (lambda ap: ap)

    # ---------------- constant / persistent pools ----------------
    const = ctx.enter_context(tc.tile_pool(name="const", bufs=1))
    small = ctx.enter_context(tc.tile_pool(name="small", bufs=1))
    bcast = ctx.enter_context(tc.tile_pool(name="bcast", bufs=1))
    pstat = ctx.enter_context(tc.tile_pool(name="pstat", bufs=2, space="PSUM"))
    pmm = ctx.enter_context(tc.tile_pool(name="pmm", bufs=2, space="PSUM"))
    dram = ctx.enter_context(tc.tile_pool(name="dram", bufs=1, space="DRAM"))

    ident = const.tile([128, 128], F32, tag="ident")
    from concourse.masks import make_identity
    make_identity(nc, ident[:])
    ones_f = const.tile([128, 1], F32, tag="ones_f")
    nc.gpsimd.memset(ones_f[:], 1.0)
    ones = const.tile([128, 1], F32, tag="ones")
    nc.vector.tensor_copy(rnd(ones[:]), ones_f[:])
    eps_t = const.tile([1, 1], F32, tag="eps")
    nc.gpsimd.memset(eps_t[:], 1e-5)

    # small parameter tiles
    def param(name, shape):
        t = const.tile(list(shape), F32, tag=name, name=name)
        nc.sync.dma_start(rnd(t[:]), rnd(io[name][:]))
        return t

    nqp = min(128, c.HC)
    bqr = param("bqr", (1, c.HC))
    bkr = param("bkr", (1, c.HC))
    bvr = param("bvr", (1, c.HC))
    wsq = param("wsq", (1, c.HC))
    wsk = param("wsk", (1, c.HC))
    wsv = param("wsv", (1, c.HC))
    bo = param("bo", (128, c.KD))
    bf1 = param("bf1", (128, c.FD))
    bf2 = param("bf2", (128, c.KD))

    inv_d = 1.0 / c.D

    def ln_stats_and_apply(xc, ncols, out, xsq_pool, tag):
        """xc: [128, KD, ncols] raw input; out: normalized, same shape."""
        ps1 = pstat.tile([1, ncols], F32, tag="st", name="ps1")
        for o in range(c.KD):
            mm(ps1, ones[:], xc[:, o, :], o == 0, o == c.KD - 1)
        ps2 = pstat.tile([1, ncols], F32, tag="st", name="ps2")
        for o in range(c.KD):
            xsq = xsq_pool.tile([128, ncols], F32, tag=f"xsq{tag}",
                                name="xsq")
            nc.vector.tensor_tensor(rnd(xsq), xc[:, o, :], xc[:, o, :], ALU.mult)
            mm(ps2, ones[:], xsq, o == 0, o == c.KD - 1)
        mu = small.tile([1, ncols], F32, tag="mu", name="mu")
        nc.vector.tensor_scalar_mul(mu, ps1, inv_d)
        ex2 = small.tile([1, ncols], F32, tag="ex2", name="ex2")
        nc.vector.tensor_scalar_mul(ex2, ps2, inv_d)
        var = small.tile([1, ncols], F32, tag="var", name="var")
        nc.vector.tensor_tensor(var, mu, mu, ALU.mult)
        nc.vector.tensor_tensor(var, ex2, var, ALU.subtract)
        std = small.tile([1, ncols], F32, tag="std", name="std")
        nc.scalar.activation(std, var, AF.Sqrt, bias=eps_t[:])
        A_ = small.tile([1, ncols], F32, tag="A", name="A_")
        nc.vector.reciprocal(A_, std)
        B_ = small.tile([1, ncols], F32, tag="B", name="B_")
        nc.vector.scalar_tensor_tensor(B_, mu, -1.0, A_, ALU.mult, ALU.mult)
        Ab = bcast.tile([128, ncols], F32, tag="Ab", name="Ab")
        nc.gpsimd.partition_broadcast(Ab, A_)
        Bb = bcast.tile([128, ncols], F32, tag="Bb", name="Bb")
        nc.gpsimd.partition_broadcast(Bb, B_)
        for o in range(c.KD):
            nc.vector.tensor_tensor(rnd(out[:, o, :]), xc[:, o, :], Ab, ALU.mult)
            nc.vector.tensor_tensor(rnd(out[:, o, :]), out[:, o, :], Bb, ALU.add)

    def ln_stats(xc, ncols, xsq_pool, tag):
        """Column stats of [128, KD, ncols] tile -> (negmu, std_, Ab), all
        rounded for f32r consumers where needed."""
        ps1 = pstat.tile([1, ncols], F32, tag="st", name="ps1")
        for o in range(c.KD):
            mm(ps1, ones[:], xc[:, o, :], o == 0, o == c.KD - 1)
        ps2 = pstat.tile([1, ncols], F32, tag="st", name="ps2")
        for o in range(c.KD):
            xsq = xsq_pool.tile([128, ncols], F32, tag=f"xsq{tag}",
                                name="xsq")
            nc.vector.tensor_tensor(rnd(xsq), xc[:, o, :], xc[:, o, :], ALU.mult)
            mm(ps2, ones[:], xsq, o == 0, o == c.KD - 1)
        mu = small.tile([1, ncols], F32, tag="mu", name="mu")
        nc.vector.tensor_scalar_mul(mu, ps1, inv_d)
        ex2 = small.tile([1, ncols], F32, tag="ex2", name="ex2")
        nc.vector.tensor_scalar_mul(ex2, ps2, inv_d)
        var = small.tile([1, ncols], F32, tag="var", name="var")
        nc.vector.tensor_tensor(var, mu, mu, ALU.mult)
        nc.vector.tensor_tensor(var, ex2, var, ALU.subtract)
        std_ = small.tile([1, ncols], F32, tag="std", name="std_")
        nc.scalar.activation(rnd(std_), var, AF.Sqrt, bias=eps_t[:])
        A_ = small.tile([1, ncols], F32, tag="A", name="A_")
        nc.vector.reciprocal(A_, std_)
        negmu = small.tile([1, ncols], F32, tag="B", name="negmu")
        nc.vector.tensor_scalar_mul(rnd(negmu), mu, -1.0)
        Ab = bcast.tile([128, ncols], F32, tag="Ab", name="Ab")
        nc.gpsimd.partition_broadcast(Ab, A_)
        return negmu, std_, Ab

    # a2a dram bounce buffers: shard j = my head-cols for (batch j//TP,
    # token block j%TP); after AllToAll block j = global head-cols
    # [j*HC, (j+1)*HC) for my (batch, token block).
    a2a_in = dram.tile([c.NC * c.HC, c.TQ], F32, tag="a2a_in")
    a2a_out = dram.tile([c.NC * c.HC, c.TQ], F32, tag="a2a_out")

    # ================= phase 1+2: LN1, QKV, attention (per batch) ==========
    with ExitStack() as ph12:
        wqkv = ph12.enter_context(tc.tile_pool(name="wqkv", bufs=1))
        xcp = ph12.enter_context(tc.tile_pool(name="xcp", bufs=2))
        xsqp = ph12.enter_context(tc.tile_pool(name="xsqp", bufs=2))
        kvqy = ph12.enter_context(tc.tile_pool(name="kvqy", bufs=1))
        sp = ph12.enter_context(tc.tile_pool(name="sp", bufs=3))
        ps_s = ph12.enter_context(tc.tile_pool(name="ps_s", bufs=2, space="PSUM"))
        ps_y = ph12.enter_context(tc.tile_pool(name="ps_y", bufs=2, space="PSUM"))

        wq = wqkv.tile([128, c.KD, c.HC], F32, tag="wq")
        nc.sync.dma_start(rnd(wq[:]), rnd(rearr(io["wq"])))
        wk = wqkv.tile([128, c.KD, c.HC], F32, tag="wk")
        nc.sync.dma_start(rnd(wk[:]), rnd(rearr(io["wk"])))
        wv = wqkv.tile([128, c.KD, c.HC], F32, tag="wv")
        nc.sync.dma_start(rnd(wv[:]), rnd(rearr(io["wv"])))

        kT = [[kvqy.tile([c.PT, c.T], F32, tag=f"kT{b}_{i}", name=f"kT{b}_{i}")
               for i in range(c.HT)] for b in range(c.B)]
        qT = [[kvqy.tile([c.PT, c.T], F32, tag=f"qT{b}_{i}", name=f"qT{b}_{i}")
               for i in range(c.HT)] for b in range(c.B)]
        yT = [[kvqy.tile([c.PT, c.T], F32, tag=f"yT{b}_{i}", name=f"yT{b}_{i}")
               for i in range(c.HT)] for b in range(c.B)]
        v_sb = [[kvqy.tile([128, c.NHC * 65], F32, tag=f"v{b}_{a}",
                           name=f"v{b}_{a}") for a in range(c.NKC)]
                for b in range(c.B)]
        for b in range(c.B):
            for a in range(c.NKC):
                nc.vector.tensor_copy(
                    rnd(v_sb[b][a][:].rearrange("p (h e) -> p h e", e=65)[:, :, 64:65]),
                    ones_f[:, 0:1].to_broadcast((128, c.NHC, 1)))

        xT = io["xT"]  # [D, B*T]
        ncq = max(1, c.HC // 128)
        for b in range(c.B):
            for t in range(c.NTL):
                tsl = slice(b * c.T + t * c.LCH, b * c.T + (t + 1) * c.LCH)
                lsl = slice(t * c.LCH, (t + 1) * c.LCH)
                xc = xcp.tile([128, c.KD, c.LCH], F32, tag="xc")
                nc.sync.dma_start(rnd(xc[:]), rnd(rearr(xT)[:, :, tsl]))
                negmu, std_, Ab = ln_stats(xc, c.LCH, xsqp, "1")

                # q / k projections on RAW x; LN folded as rank-1 terms:
                # proj = A * (x@W + (-mu) (x) wsum + std (x) bias)
                for j in range(ncq):
                    jsl = slice(j * 128, j * 128 + nqp)
                    for (wt, wst, bt, dst) in ((wq, wsq, bqr, qT),
                                               (wk, wsk, bkr, kT)):
                        ps = pmm.tile([128, c.TCH], F32, tag="mm",
                                      name="psqk")[:nqp, :c.LCH]
                        for o in range(c.KD):
                            mm(ps, wt[:, o, jsl], xc[:, o, :], o == 0, False)
                        mm(ps, wst[0:1, jsl], negmu, False, False)
                        mm(ps, bt[0:1, jsl], std_, False, True)
                        hp, r0 = (j * 128) // c.PT, (j * 128) % c.PT
                        nc.vector.tensor_tensor(rnd(dst[b][hp][r0:r0 + nqp, lsl]),
                                                ps, Ab[:nqp], ALU.mult)

                # v: feature-major matmul (N=LCH), then PE-transpose into
                # token-major v_sb tiles
                ps = pmm.tile([128, c.TCH], F32, tag="mm",
                              name="psv")[:nqp, :c.LCH]
                for o in range(c.KD):
                    mm(ps, wv[:, o, :nqp], xc[:, o, :], o == 0, False)
                mm(ps, wsv[0:1, :nqp], negmu, False, False)
                mm(ps, bvr[0:1, :nqp], std_, False, True)
                vT = sp.tile([128, c.LCH], F32, tag="vT", name="vT")[:nqp]
                nc.vector.tensor_tensor(vT, ps, Ab[:nqp], ALU.mult)
                for s2 in range(c.LCH // 128):
                    a = t * (c.LCH // 128) + s2
                    pst = pmm.tile([128, c.TCH], F32, tag="mm",
                                   name="pst")[:, :nqp]
                    nc.tensor.transpose(pst, vT[:, s2 * 128:(s2 + 1) * 128],
                                        ident[:nqp, :nqp])
                    v3 = v_sb[b][a][:].rearrange("p (h e) -> p h e",
                                                 e=65)[:, :, 0:64]
                    p3 = pst.rearrange("p (h e) -> p h e", e=64)
                    nc.vector.tensor_copy(rnd(v3), p3)

        # ---------------- attention ----------------
        isc = 1.0 / float(np.sqrt(c.HD))
        for b in range(c.B):
            for h in range(c.NHC):
                hp, m = h // c.HPT, (h % c.HPT) * 64
                rs = slice(m, m + 64)
                for qb in range(c.QB):
                    qsl = slice(qb * c.TCH, (qb + 1) * c.TCH)
                    na = (qb + 1) * c.NDIAG
                    psy = ps_y.tile([65, c.TCH], F32, tag="y", name="psy")
                    for a in range(na):
                        pss = ps_s.tile([128, c.TCH], F32, tag="s", name="pss")
                        mm(pss, kT[b][hp][rs, a * 128:(a + 1) * 128],
                           qT[b][hp][rs, qsl], True, True)
                        ssb = sp.tile([128, c.TCH], F32, tag="ssb", name="ssb")
                        d = a - qb * c.NDIAG
                        nc.scalar.activation(rnd(ssb[:]), pss[:], AF.Exp,
                                             scale=isc)
                        if d >= 0:
                            # zero the above-diagonal region in place
                            nc.gpsimd.affine_select(
                                out=rnd(ssb[:]), in_=ssb[:],
                                compare_op=ALU.is_ge, fill=0.0,
                                base=-128 * d, pattern=[[1, c.TCH]],
                                channel_multiplier=-1)
                        mm(psy, v_sb[b][a][:, h * 65:h * 65 + 65], ssb[:],
                           a == 0, a == na - 1)
                    rcp = small.tile([1, c.TCH], F32, tag="rcp", name="rcp")
                    nc.vector.reciprocal(rcp, psy[64:65, :])
                    rb = bcast.tile([64, c.TCH], F32, tag="rb", name="rb")
                    nc.gpsimd.partition_broadcast(rb[:], rcp)
                    nc.vector.tensor_tensor(yT[b][hp][rs, qsl], psy[0:64, :],
                                            rb[:], ALU.mult)

        # y -> a2a input bounce: shard j = my head cols for batch j//TP,
        # token block j%TP
        for j in range(c.NC):
            bj, tj = j // c.TP, j % c.TP
            for hp in range(c.HT):
                nc.sync.dma_start(
                    a2a_in[c.HC * j + c.PT * hp: c.HC * j + c.PT * (hp + 1), :],
                    yT[bj][hp][:, tj * c.TQ:(tj + 1) * c.TQ])

    # ================= phase 3: AllToAll over all 8 cores =================
    if getattr(c, "single", False):
        nc.sync.dma_start(a2a_out[:], a2a_in[:])
    else:
        nc.gpsimd.collective_compute(
            "AllToAll", ALU.bypass, replica_groups=[list(range(c.NC))],
            ins=[a2a_in[:].opt()], outs=[a2a_out[:].opt()])

    # ================= phase 4: out-proj, LN2, MLP =================
    with ExitStack() as ph4:
        big = ph4.enter_context(tc.tile_pool(name="big", bufs=1))
        wstr = ph4.enter_context(tc.tile_pool(name="wstr", bufs=2))
        xsqp4 = ph4.enter_context(tc.tile_pool(name="xsqp4", bufs=2))
        outp = ph4.enter_context(tc.tile_pool(name="outp", bufs=2))
        pfc2 = ph4.enter_context(tc.tile_pool(name="pfc2", bufs=1, space="PSUM"))

        oprj = ExitStack()
        oprjp = oprj.enter_context(tc.tile_pool(name="oprjp", bufs=1))
        yfull = oprjp.tile([128, c.KD, c.TQ], F32, tag="yfull")
        nc.sync.dma_start(rnd(yfull[:]), rnd(rearr(a2a_out[:])))
        xq = oprjp.tile([128, c.KD, c.TQ], F32, tag="xq")
        nc.sync.dma_start(xq[:], rearr(io["xqT"]))

        # out-projection + residual -> x2
        x2 = big.tile([128, c.KD, c.TQ], F32, tag="x2")
        for ocb in range(c.D // c.OCB):
            wo_cb = wstr.tile([128, c.KD, c.OCB], F32, tag="wbig")
            nc.sync.dma_start(rnd(wo_cb[:]),
                              rnd(rearr(io["wo"])[:, :, ocb * c.OCB:(ocb + 1) * c.OCB]))
            for j in range(c.OCB // 128):
                o = ocb * (c.OCB // 128) + j
                ps = pmm.tile([128, c.TCH], F32, tag="mm", name="pso")[:, :c.TQ]
                for k in range(c.KD):
                    mm(ps, wo_cb[:, k, j * 128:(j + 1) * 128], yfull[:, k, :],
                       k == 0, k == c.KD - 1)
                nc.vector.scalar_tensor_tensor(rnd(x2[:, o, :]), ps, bo[:, o:o + 1],
                                               xq[:, o, :], ALU.add, ALU.add)

        oprj.close()

        # LN2
        x2n = big.tile([128, c.KD, c.TQ], F32, tag="x2n")
        ln_stats_and_apply(x2, c.TQ, x2n, xsqp4, "2")

        # fc1 + gelu -> h
        h_sb = big.tile([128, c.FD, c.TQ], F32, tag="h")
        for hcb in range(c.DFF // c.HCB):
            wf1_cb = wstr.tile([128, c.KD, c.HCB], F32, tag="wbig")
            nc.sync.dma_start(rnd(wf1_cb[:]),
                              rnd(rearr(io["wf1"])[:, :, hcb * c.HCB:(hcb + 1) * c.HCB]))
            for j in range(c.HCB // 128):
                hidx = hcb * (c.HCB // 128) + j
                ps = pmm.tile([128, c.TCH], F32, tag="mm", name="psf")[:, :c.TQ]
                for o in range(c.KD):
                    mm(ps, wf1_cb[:, o, j * 128:(j + 1) * 128], x2n[:, o, :],
                       o == 0, o == c.KD - 1)
                if c.gelu_native:
                    nc.scalar.activation(rnd(h_sb[:, hidx, :]), ps, AF.Gelu_apprx_tanh,
                                         bias=bf1[:, hidx:hidx + 1])
                else:
                    # manual tanh-gelu: z=ps+b; h=0.5*z*(1+tanh(c0*(z+0.044715 z^3)))
                    z = h_sb[:, hidx, :]
                    nc.scalar.activation(z, ps, AF.Identity,
                                         bias=bf1[:, hidx:hidx + 1])
                    sq = xsqp4.tile([128, c.TCH], F32, tag="gsq",
                                    name="gsq")[:, :c.TQ]
                    nc.scalar.activation(sq, z, AF.Square)
                    nc.vector.tensor_scalar(sq, sq, 0.044715, 1.0,
                                            ALU.mult, ALU.add)
                    nc.vector.tensor_tensor(sq, sq, z, ALU.mult)
                    nc.scalar.activation(sq, sq, AF.Tanh,
                                         scale=0.7978845608028654)
                    nc.vector.tensor_scalar(sq, sq, 1.0, 0.5,
                                            ALU.add, ALU.mult)
                    nc.vector.tensor_tensor(rnd(z), z, sq, ALU.mult)

        # fc2 + residual -> out
        for dcb in range(c.D // c.OCB):
            nb = c.OCB // 128
            psums = [pfc2.tile([128, c.TQ], F32, tag=f"fc2_{i}", name=f"fc2_{i}")
                     for i in range(nb)]
            KHB = min(8, c.FD)
            wf2r = io["wf2"].rearrange("(o p) d -> p o d", p=128)
            for khb in range(c.FD // KHB):
                wf2_t = wstr.tile([128, KHB, c.OCB], F32, tag="wbig")
                nc.sync.dma_start(
                    rnd(wf2_t[:]),
                    rnd(wf2r[:, khb * KHB:(khb + 1) * KHB,
                             dcb * c.OCB:(dcb + 1) * c.OCB]))
                for k2 in range(KHB):
                    kh = khb * KHB + k2
                    for j in range(nb):
                        mm(psums[j], wf2_t[:, k2, j * 128:(j + 1) * 128],
                           h_sb[:, kh, :], kh == 0, kh == c.FD - 1)
            for j in range(nb):
                o = dcb * nb + j
                ot = outp.tile([128, c.TQ], F32, tag="ot", name="ot")
                nc.vector.scalar_tensor_tensor(ot[:], psums[j], bf2[:, o:o + 1],
                                               x2[:, o, :], ALU.add, ALU.add)
                nc.sync.dma_start(rearr(io["out"])[:, o, :], ot[:])


# ---------------- host-side sharding ----------------

def pack_pf(v, D):
    """[D] per-feature vector -> [128, D//128] with [p, o] = v[128*o + p]."""
    return np.ascontiguousarray(np.asarray(v, np.float32).reshape(D // 128, 128).T)


def make_in_maps(inputs, cfg):
    c = cfg
    x = np.asarray(inputs["x"], np.float32)
    w_qkv = np.asarray(inputs["w_qkv"], np.float32)
    b_qkv = np.asarray(inputs["b_qkv"], np.float32)
    w_o = np.ascontiguousarray(np.asarray(inputs["w_o"], np.float32))
    w_fc1 = np.ascontiguousarray(np.asarray(inputs["w_fc1"], np.float32))
    w_fc2 = np.ascontiguousarray(np.asarray(inputs["w_fc2"], np.float32))
    D = c.D

    xT_all = np.concatenate([x[b].T for b in range(c.B)], axis=1)
    xT_all = np.ascontiguousarray(xT_all)  # [D, B*T]

    # fold LN affine into projection weights: LN(x) = xn0*g + b with
    # xn0=(x-mu)/std; xn0 @ (g*W) + (b@W + bias) == LN(x) @ W + bias
    g1 = np.asarray(inputs["ln1_g"], np.float32)
    b1 = np.asarray(inputs["ln1_b"], np.float32)
    g2 = np.asarray(inputs["ln2_g"], np.float32)
    b2 = np.asarray(inputs["ln2_b"], np.float32)
    w_qkv_f = w_qkv * g1[:, None]
    b_qkv_f = b_qkv + b1 @ w_qkv
    w_fc1_f = np.ascontiguousarray(w_fc1 * g2[:, None])
    b_fc1_f = np.asarray(inputs["b_fc1"], np.float32) + b2 @ w_fc1

    in_maps = []
    for core in range(c.NC):
        b, p = core // c.TP, core % c.TP
        hc0 = core * c.HC                   # global head-col base of this core
        qs, ks, vs = hc0, D + hc0, 2 * D + hc0
        rows = slice(p * c.TQ, (p + 1) * c.TQ)
        nqp = min(128, c.HC)
        m = {
            "xT": xT_all,
            "xqT": np.ascontiguousarray(x[b, rows, :].T),
            "wq": np.ascontiguousarray(w_qkv_f[:, qs:qs + c.HC]),
            "wk": np.ascontiguousarray(w_qkv_f[:, ks:ks + c.HC]),
            "wv": np.ascontiguousarray(w_qkv_f[:, vs:vs + c.HC]),
            "bqr": np.ascontiguousarray(b_qkv_f[None, qs:qs + c.HC]),
            "bkr": np.ascontiguousarray(b_qkv_f[None, ks:ks + c.HC]),
            "bvr": np.ascontiguousarray(b_qkv_f[None, vs:vs + c.HC]),
            "wsq": np.ascontiguousarray(
                w_qkv_f[:, qs:qs + c.HC].sum(0, keepdims=True)),
            "wsk": np.ascontiguousarray(
                w_qkv_f[:, ks:ks + c.HC].sum(0, keepdims=True)),
            "wsv": np.ascontiguousarray(
                w_qkv_f[:, vs:vs + c.HC].sum(0, keepdims=True)),
            "wo": w_o,
            "bo": pack_pf(inputs["b_o"], D),
            "wf1": w_fc1_f,
            "bf1": pack_pf(b_fc1_f, c.DFF),
            "wf2": w_fc2,
            "bf2": pack_pf(inputs["b_fc2"], D),
        }
        in_maps.append(m)
    return in_maps


def assemble_output(results, cfg):
    c = cfg
    out = np.empty((c.B, c.T, c.D), np.float32)
    for core in range(c.NC):
        b, p = core // c.TP, core % c.TP
        out[b, p * c.TQ:(p + 1) * c.TQ, :] = results[core]["out"].T
    return out


def build_nc(cfg):
    nc = bacc.Bacc("TRN2", target_bir_lowering=False, debug=False,
                   num_devices=cfg.NC, name="nn_block")
    c = cfg
    io = {}
    specs = {
        "xT": (c.D, c.B * c.T), "xqT": (c.D, c.TQ),
        "wq": (c.D, c.HC), "wk": (c.D, c.HC), "wv": (c.D, c.HC),
        "bqr": (1, c.HC), "bkr": (1, c.HC), "bvr": (1, c.HC),
        "wsq": (1, c.HC), "wsk": (1, c.HC), "wsv": (1, c.HC),
        "wo": (c.D, c.D), "bo": (128, c.KD),
        "wf1": (c.D, c.DFF), "bf1": (128, c.FD),
        "wf2": (c.DFF, c.D), "bf2": (128, c.KD),
    }
    for name, shape in specs.items():
        io[name] = nc.declare_dram_parameter(name, list(shape), F32,
                                             isOutput=False).ap()
    io["out"] = nc.declare_dram_parameter("out", [c.D, c.TQ], F32,
                                          isOutput=True).ap()
    with tile.TileContext(nc) as tc:
        with ExitStack() as ctx:
            emit(ctx, tc, io, cfg)
    nc.compile()
    return nc


_CACHE = {}


def kernel(**inputs) -> np.ndarray:
    from concourse.bass_utils import run_bass_kernel_spmd
    cfg = CFG
    if "nc" not in _CACHE:
        _CACHE["nc"] = build_nc(cfg)
    nc = _CACHE["nc"]
    in_maps = make_in_maps(inputs, cfg)
    res = run_bass_kernel_spmd(nc, in_maps, core_ids=list(range(cfg.NC)))
    return assemble_output(res.results, cfg)
